# revision 40
# baseline (speedup 1.0000x reference)
"""Cross-attention (txt queries -> image kv) Trainium2 Bass kernel.

Sharding: data-parallel over batch — B=8 batches, one NeuronCore each.
Host-side prep: image columns are COMPACTED to valid kv positions (padded to
jp, a multiple of 128) and txt rows PERMUTED valid-first (attention runs on
the first ip columns only; outputs un-permuted on host). Invalid-q rows are
reconstructed exactly via the ymeanb blend (uniform attention over all kv).
Per core (batch b):
  Q^T = Wq^T T^T / 8          [e, i]   bf16 SBUF-resident
  ONE dense stream over imT (per 512-col block): V' = (X Wv) * kvm_j [j, e]
    SBUF-resident as [j, jc, h, 65] tiles whose col 64 holds kvm_j (so PV
    accumulates both the numerator and the softmax denominator with the kv
    mask applied exactly), and K^T = Wk^T X^T [e, j] SBUF-resident — K and V
    share each image block load; one long back-to-back PE burst keeps the
    HAM clock-gate warm.
  Then attention per head-pair hp (2 heads per psum tile):
    S^T_h = K_h Q_h^T         [j, i]   psum [128,2,512]
    P^T = exp(S^T)            (padded j columns have K^T=0 -> exp(0)=1, and
                               are killed by V'=0 / kvm denominator row)
    O^T_aug_h = [V'_h | kvm]^T P^T -> [65, i] psum accum over j chunks
    O^T = O^T_aug[0:64] * recip(denom) broadcast via PE ones-outer-product
  Y = O^T.T Wout; blend: y = qm_i*Y + (1-qm_i)*ymeanb + qm_i*bout
    (q_mask=False rows = uniform attention over all kv -> host-computed
     ymeanb = (mean_j X) @ Wv @ Wout + bout)

All PE matmuls in bf16 (inputs pre-cast to bf16 on host; intermediates cast
during psum->sbuf copies). Accumulation is fp32 in PSUM throughout. fp8
(e4m3) projection paths exist behind FP8_* flags but are OFF: measured
end-to-end rel-err 1.7e-2 (K) to 3.4e-2 (K+V) vs the 2e-2 gate.

build_nc(reps=N) wraps the whole body in an on-device For_i: test.py times
reps=1 vs reps=33 dispatches and differences them to remove the ~80 ms
axon-relay floor from the reported HW exec time.
"""

import ml_dtypes
import numpy as np

import concourse.bass as bass
from concourse import bacc
import concourse.mybir as mybir
import concourse.tile as tile
from concourse.bass_utils import run_bass_kernel_spmd

B, I, J, D, E = 8, 512, 4096, 1024, 1024
H, DH = 16, 64
JC = J // 128   # 32
IC = I // 128   # 4
DC = D // 128   # 8
EC = E // 128   # 8
F32 = mybir.dt.float32
BF16 = mybir.dt.bfloat16
BF = ml_dtypes.bfloat16

# Structure switch: False = dense sequential phases (KV stream once, then
# attention) — keeps PE streams long for the HAM clock-gate. True = 4-pass
# flash-fused variant (better sim, measured worse on HW).
FUSED = False
# fp8 (e4m3) DoubleRow for the projection matmuls: image/txt activations in
# fp8 directly (values ~N(0,1)); weights pre-scaled by WSCALE=64 on host so
# they sit in e4m3's normal range, descaled when psum is read back.
FP8_KV = False   # K and V projections
FP8_Q = False    # Q projection
D_INWINDOW = False  # tail-D measured better: in-window D partials
# steal 's2' psum slots from QK, shrinking ACT's lookahead cushion and
# stretching the attention window ~1:1 (sim 259.8us vs 248.5us)
F8 = mybir.dt.float8e4
NP8 = ml_dtypes.float8_e4m3
WSCALE = 64.0


def build_nc(jp=J, ip=I, reps=1):
    import contextlib

    jcp = jp // 128
    nc = bacc.Bacc()

    txtT = nc.dram_tensor("txtT", [D, I], F8 if FP8_Q else BF16, kind="ExternalInput")
    imT = nc.dram_tensor("imT", [D, jp], F8 if FP8_KV else BF16, kind="ExternalInput")
    wq = nc.dram_tensor("wq", [D, E], F8 if FP8_Q else BF16, kind="ExternalInput")
    wkv = nc.dram_tensor("wkv", [D, 2 * E], F8 if FP8_KV else BF16, kind="ExternalInput")
    wout = nc.dram_tensor("wout", [E, D], BF16, kind="ExternalInput")
    kvmp = nc.dram_tensor("kvmp", [128, jcp], F32, kind="ExternalInput")
    qmp = nc.dram_tensor("qmp", [128, IC], F32, kind="ExternalInput")
    qmrow = nc.dram_tensor("qmrow", [1, I], BF16, kind="ExternalInput")
    omqrow = nc.dram_tensor("omqrow", [1, I], BF16, kind="ExternalInput")
    ymeanb = nc.dram_tensor("ymeanb", [1, D], BF16, kind="ExternalInput")
    boutr = nc.dram_tensor("boutr", [1, D], BF16, kind="ExternalInput")
    y = nc.dram_tensor("y", [I, D], F32, kind="ExternalOutput")
    # V round-trip buffer: [jc, p, h, dh] so phase-B writes are contiguous
    vdr = nc.dram_tensor("vdr", [jcp, 128, H, DH], BF16, kind="Internal")

    imTr = imT[:].rearrange("(dc p) j -> p dc j", p=128)

    with tile.TileContext(nc) as tc:
        with (
            tc.tile_pool(name="wpool", bufs=1) as wpool,      # small resident
            tc.tile_pool(name="wslot", bufs=2) as wslot,      # wk+wv live together in fused pass
            tc.tile_pool(name="big", bufs=1) as big,          # K^T, Q^T, O^T
            tc.tile_pool(name="streamq", bufs=1) as streamq,  # txtT (phase Q only)
            tc.tile_pool(name="stream", bufs=2) as stream,    # imT/vt blocks
            tc.tile_pool(name="work", bufs=3) as work,        # small working tiles
            tc.tile_pool(name="etp", bufs=9) as etp,
            tc.tile_pool(
                name="ppool", bufs=(2 if FUSED else 3), space="PSUM"
            ) as ppool,
            tc.tile_pool(
                name="accp", bufs=(4 if FUSED else 2), space="PSUM"
            ) as accp,
            tc.For_i(
                0, reps, 1,
                hint_engines=(
                    mybir.EngineType.PE,
                    mybir.EngineType.Activation,
                    mybir.EngineType.DVE,
                    mybir.EngineType.Pool,
                    mybir.EngineType.SP,
                ),
            ) if reps > 1 else contextlib.nullcontext(),
        ):
            # big phase-Q inputs FIRST so their transfers head the DMA order
            ta = streamq.tile([128, DC, I], F8 if FP8_Q else BF16, tag="im")
            nc.gpsimd.dma_start(ta[:], txtT[:].rearrange("(dc p) i -> p dc i", p=128))
            wq_sb = wslot.tile([128, DC, E], F8 if FP8_Q else BF16, tag="w")
            nc.gpsimd.dma_start(
                wq_sb[:, :, 0 : E // 2],
                wq[:, 0 : E // 2].rearrange("(dc p) e -> p dc e", p=128),
            )
            nc.gpsimd.dma_start(
                wq_sb[:, :, E // 2 : E],
                wq[:, E // 2 : E].rearrange("(dc p) e -> p dc e", p=128),
            )

            # ---- resident small inputs ----
            kvm_sb = wpool.tile([128, jcp], F32)
            nc.sync.dma_start(kvm_sb[:], kvmp[:])
            qmp_sb = wpool.tile([128, IC], F32)
            nc.sync.dma_start(qmp_sb[:], qmp[:])
            qmrow_sb = wpool.tile([1, I], BF16)
            nc.sync.dma_start(qmrow_sb[:], qmrow[:])
            omqrow_sb = wpool.tile([1, I], BF16)
            nc.sync.dma_start(omqrow_sb[:], omqrow[:])
            ymeanb_sb = wpool.tile([1, D], BF16)
            nc.sync.dma_start(ymeanb_sb[:], ymeanb[:])
            boutr_sb = wpool.tile([1, D], BF16)
            nc.sync.dma_start(boutr_sb[:], boutr[:])
            ones64 = wpool.tile([1, DH], BF16)
            nc.vector.memset(ones64[:], 1.0)
            if FP8_KV:
                kvm_v = wpool.tile([128, jcp], F32)
                nc.vector.tensor_scalar_mul(kvm_v[:], kvm_sb[:], 1.0 / WSCALE)
            else:
                kvm_v = kvm_sb

            KT_sb = big.tile([128, EC, jp], BF16)
            QT_sb = big.tile([128, EC, I], BF16)          # 8 KB/part
            OT_sb = big.tile([128, EC, I], BF16)
            nc.vector.memset(OT_sb[:], 0.0)

            # ============ Phase Q: Q^T = Wq^T @ T^T, scaled 1/8 ==========
            for ep in range(EC // 2):
                ps = ppool.tile([128, 2, 512], F32, tag="s2")
                for eh in range(2):
                    ec = 2 * ep + eh
                    if FP8_Q:
                        for dc in range(0, DC, 2):
                            nc.tensor.matmul(
                                ps[:, eh, 0:ip],
                                wq_sb[:, dc : dc + 2, ec * 128 : (ec + 1) * 128],
                                ta[:, dc : dc + 2, 0:ip],
                                start=(dc == 0),
                                stop=(dc == DC - 2),
                                perf_mode=mybir.MatmulPerfMode.DoubleRow,
                            )
                    else:
                        for dc in range(DC):
                            nc.tensor.matmul(
                                ps[:, eh, 0:ip],
                                wq_sb[:, dc, ec * 128 : (ec + 1) * 128],
                                ta[:, dc, 0:ip],
                                start=(dc == 0),
                                stop=(dc == DC - 1),
                            )
                nc.vector.tensor_scalar_mul(
                    QT_sb[:, 2 * ep : 2 * ep + 2, 0:ip],
                    ps[:, :, 0:ip],
                    0.125 / (WSCALE if FP8_Q else 1.0),
                )

            # Padded j columns have K^T = 0 (image compacted+zero-padded) so
            # S = 0 there and exp(0)=1; both numerator and denominator kill
            # them via V'=0 / kvm row. No mask needed inside exp.
            v_res = jp <= 2560

            jblocks = []
            off = 0
            while off < jp:
                w = 512 if jp - off >= 512 else jp - off
                jblocks.append((off, w))
                off += w

            def epilogue(hp, hh, oacc):
                # 1/denom broadcast along partitions via the GPSIMD
                # partition_broadcast custom op — Pool is idle during
                # attention and this keeps the epilogue entirely off PE.
                rec = work.tile([1, ip], F32, tag="rec")
                nc.vector.reciprocal(rec[:, 0:ip], oacc[DH : DH + 1, 0:ip])
                rb = work.tile([DH, ip], F32, tag="rb")
                nc.gpsimd.partition_broadcast(rb[:, 0:ip], rec[:, 0:ip])
                nc.vector.tensor_tensor(
                    OT_sb[hh * DH : (hh + 1) * DH, hp, 0:ip],
                    oacc[0:DH, 0:ip],
                    rb[:, 0:ip],
                    mybir.AluOpType.mult,
                )

            def qk_exp(hp, jc):
                sps = ppool.tile([128, 2, 512], F32, tag="s2")
                nc.tensor.matmul(
                    sps[:, 0, 0:ip],
                    KT_sb[0:DH, hp, jc * 128 : (jc + 1) * 128],
                    QT_sb[0:DH, hp, 0:ip],
                    start=True,
                    stop=True,
                )
                nc.tensor.matmul(
                    sps[:, 1, 0:ip],
                    KT_sb[DH:128, hp, jc * 128 : (jc + 1) * 128],
                    QT_sb[DH:128, hp, 0:ip],
                    start=True,
                    stop=True,
                )
                et = etp.tile([128, 2, ip], BF16, tag="et")
                nc.scalar.activation(
                    et[:, :, 0:ip],
                    sps[:, :, 0:ip],
                    mybir.ActivationFunctionType.Exp,
                )
                return et

            def pv(jc, vt, et, oacc_a, oacc_b):
                nc.tensor.matmul(
                    oacc_a[:, 0:ip],
                    vt[:, jc, 0, :],
                    et[:, 0, 0:ip],
                    start=(jc == 0),
                    stop=(jc == jcp - 1),
                )
                nc.tensor.matmul(
                    oacc_b[:, 0:ip],
                    vt[:, jc, 1, :],
                    et[:, 1, 0:ip],
                    start=(jc == 0),
                    stop=(jc == jcp - 1),
                )

            def qk_exp_pv(hp, jc, vt, oacc_a, oacc_b):
                et = qk_exp(hp, jc)
                pv(jc, vt, et, oacc_a, oacc_b)

            def k_block(ep, imb, off, w):
                ps = ppool.tile([128, 2, 512], F32, tag="s2")
                for eh in range(2):
                    ec = 2 * ep + eh
                    if FP8_KV:
                        for dc in range(0, DC, 2):
                            nc.tensor.matmul(
                                ps[:, eh, 0:w],
                                wk_sb[:, dc : dc + 2, ec * 128 : (ec + 1) * 128],
                                imb[:, dc : dc + 2, 0:w],
                                start=(dc == 0),
                                stop=(dc == DC - 2),
                                perf_mode=mybir.MatmulPerfMode.DoubleRow,
                            )
                    else:
                        for dc in range(DC):
                            nc.tensor.matmul(
                                ps[:, eh, 0:w],
                                wk_sb[:, dc, ec * 128 : (ec + 1) * 128],
                                imb[:, dc, 0:w],
                                start=(dc == 0),
                                stop=(dc == DC - 1),
                            )
                if FP8_KV:
                    nc.vector.tensor_scalar_mul(
                        KT_sb[:, 2 * ep : 2 * ep + 2, off : off + w],
                        ps[:, :, 0:w],
                        1.0 / WSCALE,
                    )
                else:
                    nc.vector.tensor_copy(
                        KT_sb[:, 2 * ep : 2 * ep + 2, off : off + w], ps[:, :, 0:w]
                    )

            def v_block(imb, off, w):
                for jh in range(w // 128):
                    jc = off // 128 + jh
                    ps = ppool.tile([128, 2, 512], F32, tag="s2")
                    for eb in range(2):
                        if FP8_KV:
                            for dc in range(0, DC, 2):
                                nc.tensor.matmul(
                                    ps[:, eb, :],
                                    imb[:, dc : dc + 2, jh * 128 : (jh + 1) * 128],
                                    wv_sb[:, dc : dc + 2, eb * 512 : (eb + 1) * 512],
                                    start=(dc == 0),
                                    stop=(dc == DC - 2),
                                    perf_mode=mybir.MatmulPerfMode.DoubleRow,
                                )
                        else:
                            for dc in range(DC):
                                nc.tensor.matmul(
                                    ps[:, eb, :],
                                    imb[:, dc, jh * 128 : (jh + 1) * 128],
                                    wv_sb[:, dc, eb * 512 : (eb + 1) * 512],
                                    start=(dc == 0),
                                    stop=(dc == DC - 1),
                                )
                    if v_res:
                        nc.vector.tensor_scalar_mul(
                            V_sb[:, jc, :, 0:DH],
                            ps[:].rearrange("p b (h dh) -> p (b h) dh", dh=DH),
                            kvm_v[:, jc : jc + 1],
                        )
                    else:
                        vtmp = work.tile([128, H, DH], BF16, tag="vtmp")
                        nc.vector.tensor_scalar_mul(
                            vtmp[:],
                            ps[:].rearrange("p b (h dh) -> p (b h) dh", dh=DH),
                            kvm_v[:, jc : jc + 1],
                        )
                        nc.sync.dma_start(vdr[jc, :, :, :], vtmp[:])

            if v_res and not FUSED:
                # === v2: dense sequential phases. ONE stream over imT
                # producing V (resident) + all of K^T back-to-back (long PE
                # burst, HAM-friendly), then attention per head-pair.
                # (Measured best on HW: 309.9us vs 326.4us for a two-stream
                # K/attention pipeline and 363.1us for per-block fusion.) ===
                V_sb = big.tile([128, jcp, H, DH + 1], BF16)
                nc.vector.tensor_copy(
                    V_sb[:, :, :, DH : DH + 1],
                    kvm_sb[:, :, None, None].to_broadcast([128, jcp, H, 1]),
                )
                imb0 = stream.tile([128, DC, 512], F8 if FP8_KV else BF16, tag="ima")
                nc.gpsimd.dma_start(
                    imb0[:, :, 0 : jblocks[0][1]],
                    imTr[:, :, 0 : jblocks[0][1]],
                )
                wv_sb = wslot.tile([128, DC, E], F8 if FP8_KV else BF16, tag="w")
                nc.gpsimd.dma_start(
                    wv_sb[:], wkv[:, E : 2 * E].rearrange("(dc p) e -> p dc e", p=128)
                )
                wk_sb = wslot.tile([128, DC, E], F8 if FP8_KV else BF16, tag="w")
                nc.gpsimd.dma_start(
                    wk_sb[:], wkv[:, 0:E].rearrange("(dc p) e -> p dc e", p=128)
                )
                for bi, (off, w) in enumerate(jblocks):
                    if bi == 0:
                        imb = imb0
                    else:
                        imb = stream.tile([128, DC, 512], F8 if FP8_KV else BF16, tag="ima")
                        nc.gpsimd.dma_start(
                            imb[:, :, 0:w], imTr[:, :, off : off + w]
                        )
                    v_block(imb, off, w)
                    for ep in range(EC // 2):
                        k_block(ep, imb, off, w)
                icv = (ip + 127) // 128
                if D_INWINDOW:
                    wo_sb = wslot.tile([128, DC, E], BF16, tag="w")
                    nc.scalar.dma_start(
                        wo_sb[:], wout[:].rearrange("(ec p) d -> p ec d", p=128)
                    )
                    y_acc = big.tile([128, icv, 2, 512], F32)

                def d_partial_ic(hp, ic):
                    ps = ppool.tile([128, 2, 512], F32, tag="s2")
                    for db in range(2):
                        nc.tensor.matmul(
                            ps[:, db, :],
                            OT_sb[:, hp, ic * 128 : (ic + 1) * 128],
                            wo_sb[:, hp, db * 512 : (db + 1) * 512],
                            start=True,
                            stop=True,
                        )
                    if hp == 0:
                        nc.vector.tensor_copy(y_acc[:, ic, :, :], ps[:])
                    else:
                        nc.vector.tensor_tensor(
                            y_acc[:, ic, :, :],
                            ps[:],
                            y_acc[:, ic, :, :],
                            mybir.AluOpType.add,
                        )

                # epilogue/d_partial for head-pair hp are EMITTED a few jc
                # iterations into head-pair hp+1 so the PE queue never
                # head-of-line blocks on the epilogue's DVE chain — ACT keeps
                # streaming exps across the boundary.
                # Deferred work from head-pair hp-1 (epilogues, phase-D
                # partial out-projections) is dripped into hp's jc loop one
                # piece every other iteration: each dent in the PE queue must
                # stay under the ~2-slot sps cushion or ACT's exp stream
                # hiccups. PV trails QK/exp by one step, carried across
                # boundaries, so the in-order PE queue never blocks on exp.
                pend = []
                prev = None  # (jc, vt, et, oacc_a, oacc_b)
                for hp in range(EC):
                    vt = V_sb[:, :, 2 * hp : 2 * hp + 2, :]
                    oacc_a = accp.tile([DH + 1, 512], F32, tag="oacc")
                    oacc_b = accp.tile([DH + 1, 512], F32, tag="oacc")
                    for jc in range(jcp):
                        et = qk_exp(hp, jc)
                        if prev is not None:
                            pv(*prev)
                        prev = (jc, vt, et, oacc_a, oacc_b)
                        if jc >= 3 and jc % 2 == 1 and pend:
                            pend.pop(0)()
                    # drain leftovers (only fires for tiny jcp) before
                    # queueing this head-pair's deferred work
                    for f in pend:
                        f()
                    ph, pa, pb = hp, oacc_a, oacc_b
                    pend = [
                        lambda ph=ph, pa=pa: epilogue(ph, 0, pa),
                        lambda ph=ph, pb=pb: epilogue(ph, 1, pb),
                    ]
                    if D_INWINDOW:
                        pend += [
                            lambda ph=ph: d_partial_ic(ph, 0),
                            lambda ph=ph: d_partial_ic(ph, 1),
                            lambda ph=ph: d_partial_ic(ph, 2),
                        ][:icv]
                pv(*prev)
                for f in pend:
                    f()
            elif v_res:
                # === v3: 4 fused passes over imT. Pass p computes K^T for ec
                # chunks {2p, 2p+1} (+ V on pass 0) and, block by block, runs
                # QK->exp->PV for head-pairs 2p and 2p+1; exp (ACT) hides
                # under the projection matmuls (PE). PV accumulates in PSUM
                # across the whole stream (4 banks live per pass).
                V_sb = big.tile([128, jcp, H, DH + 1], BF16)
                nc.vector.tensor_copy(
                    V_sb[:, :, :, DH : DH + 1],
                    kvm_sb[:, :, None, None].to_broadcast([128, jcp, H, 1]),
                )
                # block-0 image lands before wv/wk so pass-0 V can start early
                imb0 = stream.tile([128, DC, 512], F8 if FP8_KV else BF16, tag="ima")
                nc.gpsimd.dma_start(
                    imb0[:, :, 0 : jblocks[0][1]],
                    imTr[:, :, 0 : jblocks[0][1]],
                )
                wv_sb = wslot.tile([128, DC, E], F8 if FP8_KV else BF16, tag="w")
                nc.gpsimd.dma_start(
                    wv_sb[:], wkv[:, E : 2 * E].rearrange("(dc p) e -> p dc e", p=128)
                )
                wk_sb = wslot.tile([128, DC, E], F8 if FP8_KV else BF16, tag="w")
                nc.gpsimd.dma_start(
                    wk_sb[:], wkv[:, 0:E].rearrange("(dc p) e -> p dc e", p=128)
                )
                for p in range(EC // 2):
                    oaccs = [
                        [
                            accp.tile(
                                [DH + 1, 512], F32, tag="oacc",
                                name=f"oacc_{p}_{hl}_{hh}",
                            )
                            for hh in range(2)
                        ]
                        for hl in range(2)
                    ]  # [hp_local][hh]

                    def attn_block(off, w):
                        for hl in range(2):
                            hp = 2 * p + hl
                            vt = V_sb[:, :, 2 * hp : 2 * hp + 2, :]
                            for jh in range(w // 128):
                                qk_exp_pv(
                                    hp,
                                    off // 128 + jh,
                                    vt,
                                    oaccs[hl][0],
                                    oaccs[hl][1],
                                )

                    # one-block software pipeline: attention for block b-1
                    # runs while block b's K projection streams, hiding the
                    # psum->KT copy latency.
                    prev = None
                    for bi, (off, w) in enumerate(jblocks):
                        if p == 0 and bi == 0:
                            imb = imb0
                        else:
                            imb = stream.tile([128, DC, 512], F8 if FP8_KV else BF16, tag="ima")
                            nc.gpsimd.dma_start(
                                imb[:, :, 0:w], imTr[:, :, off : off + w]
                            )
                        if p == 0:
                            v_block(imb, off, w)
                        k_block(p, imb, off, w)
                        if prev is not None:
                            attn_block(*prev)
                        prev = (off, w)
                    attn_block(*prev)
                    for hl in range(2):
                        for hh in range(2):
                            epilogue(2 * p + hl, hh, oaccs[hl][hh])
            else:
                # === fallback (large jp): single fused K+V pass with V via
                # DRAM round-trip, then attention per head-pair ===
                wv_sb = wslot.tile([128, DC, E], F8 if FP8_KV else BF16, tag="w")
                nc.gpsimd.dma_start(
                    wv_sb[:], wkv[:, E : 2 * E].rearrange("(dc p) e -> p dc e", p=128)
                )
                wk_sb = wslot.tile([128, DC, E], F8 if FP8_KV else BF16, tag="w")
                nc.gpsimd.dma_start(
                    wk_sb[:], wkv[:, 0:E].rearrange("(dc p) e -> p dc e", p=128)
                )
                for off, w in jblocks:
                    imb = stream.tile([128, DC, 512], F8 if FP8_KV else BF16, tag="ima")
                    nc.gpsimd.dma_start(imb[:, :, 0:w], imTr[:, :, off : off + w])
                    v_block(imb, off, w)
                    for ep in range(EC // 2):
                        k_block(ep, imb, off, w)
                for hp in range(EC):
                    vtt = stream.tile([128, jcp, 2, DH + 1], BF16, tag="vt")
                    nc.vector.tensor_copy(
                        vtt[:, :, 0, DH : DH + 1], kvm_sb[:, :, None]
                    )
                    nc.vector.tensor_copy(
                        vtt[:, :, 1, DH : DH + 1], kvm_sb[:, :, None]
                    )
                    for hh in range(2):
                        nc.sync.dma_start(
                            vtt[:, :, hh, 0:DH],
                            vdr[:, :, 2 * hp + hh, :].rearrange("jc p dh -> p jc dh"),
                        )
                    oacc_a = accp.tile([DH + 1, 512], F32, tag="oacc")
                    oacc_b = accp.tile([DH + 1, 512], F32, tag="oacc")
                    for jc in range(jcp):
                        qk_exp_pv(hp, jc, vtt, oacc_a, oacc_b)
                    epilogue(hp, 0, oacc_a)
                    epilogue(hp, 1, oacc_b)

            # ============ Phase D tail: blend + store ====================
            have_yacc = v_res and not FUSED and D_INWINDOW
            if not have_yacc:
                wo_sb = wslot.tile([128, DC, E], BF16, tag="w")
                nc.scalar.dma_start(
                    wo_sb[:], wout[:].rearrange("(ec p) d -> p ec d", p=128)
                )
            for ic in range(IC):
                has_valid = ic * 128 < ip
                if has_valid and not have_yacc:
                    yps = ppool.tile([128, 2, 512], F32, tag="s2")
                    for db in range(2):
                        for ec in range(EC):
                            nc.tensor.matmul(
                                yps[:, db, :],
                                OT_sb[:, ec, ic * 128 : (ic + 1) * 128],
                                wo_sb[:, ec, db * 512 : (db + 1) * 512],
                                start=(ec == 0),
                                stop=(ec == EC - 1),
                            )
                bb_a = accp.tile([128, 512], F32, tag="oacc")
                bb_b = accp.tile([128, 512], F32, tag="oacc")
                for db, bbps in ((0, bb_a), (1, bb_b)):
                    nc.tensor.matmul(
                        bbps[:],
                        omqrow_sb[:, ic * 128 : (ic + 1) * 128],
                        ymeanb_sb[:, db * 512 : (db + 1) * 512],
                        start=True,
                        stop=False,
                    )
                    nc.tensor.matmul(
                        bbps[:],
                        qmrow_sb[:, ic * 128 : (ic + 1) * 128],
                        boutr_sb[:, db * 512 : (db + 1) * 512],
                        start=False,
                        stop=True,
                    )
                y1 = work.tile([128, 2, 512], F32, tag="y1")
                if has_valid:
                    nc.vector.tensor_scalar_mul(
                        y1[:],
                        y_acc[:, ic, :, :] if have_yacc else yps[:],
                        qmp_sb[:, ic : ic + 1],
                    )
                    nc.vector.tensor_tensor(
                        y1[:, 0, :], bb_a[:], y1[:, 0, :], mybir.AluOpType.add
                    )
                    nc.vector.tensor_tensor(
                        y1[:, 1, :], bb_b[:], y1[:, 1, :], mybir.AluOpType.add
                    )
                else:
                    nc.vector.tensor_copy(y1[:, 0, :], bb_a[:])
                    nc.vector.tensor_copy(y1[:, 1, :], bb_b[:])
                nc.sync.dma_start(
                    y[ic * 128 : (ic + 1) * 128, :],
                    y1[:].rearrange("p b d -> p (b d)"),
                )

    nc.compile()
    return nc


_NC_CACHE = {}


def _get_nc(jp=J, ip=I, reps=1):
    key = (jp, ip, reps)
    if key not in _NC_CACHE:
        _NC_CACHE[key] = build_nc(jp, ip, reps)
    return _NC_CACHE[key]


def prep_inputs(txt, image, kv_mask, q_mask, Wq, Wkv, Wout, bout):
    f32 = np.float32
    Wq = np.asarray(Wq, dtype=f32)
    Wkv = np.asarray(Wkv, dtype=f32)
    Wout = np.asarray(Wout, dtype=f32)
    bout = np.asarray(bout, dtype=f32)
    wq_b = Wq.astype(BF)
    wkv_b = Wkv.astype(BF)
    wout_b = Wout.astype(BF)
    kvc = kv_mask.sum(axis=1).max()
    qc = q_mask.sum(axis=1).max()
    jp = max(512, int(-(-kvc // 128)) * 128)
    ip = max(256, int(-(-qc // 16)) * 16)
    jcp = jp // 128
    in_maps = []
    perms = []
    for b in range(B):
        kvm = kv_mask[b].astype(bool)
        qm = q_mask[b].astype(bool)
        nkv = int(kvm.sum())
        # compact image columns to valid kv positions, zero-pad to jp
        imTc = np.zeros((D, jp), dtype=BF)
        imTc[:, :nkv] = np.ascontiguousarray(image[b][kvm].T).astype(BF)
        kvmp = np.zeros(jp, dtype=f32)
        kvmp[:nkv] = 1.0
        # permute txt rows valid-first
        perm = np.argsort(~qm, kind="stable")
        perms.append(perm)
        qmperm = qm[perm].astype(f32)
        xmean = image[b].astype(f32).mean(axis=0)
        vmean = xmean @ Wkv[:, E:]
        ymb = vmean @ Wout + bout
        in_maps.append(
            {
                "txtT": np.ascontiguousarray(txt[b][perm].T).astype(BF),
                "imT": imTc,
                "wq": wq_b,
                "wkv": wkv_b,
                "wout": wout_b,
                "kvmp": np.ascontiguousarray(kvmp.reshape(jcp, 128).T),
                "qmp": np.ascontiguousarray(qmperm.reshape(IC, 128).T),
                "qmrow": qmperm[None, :].astype(BF),
                "omqrow": (1.0 - qmperm)[None, :].astype(BF),
                "ymeanb": ymb[None, :].astype(BF),
                "boutr": bout[None, :].astype(BF),
            }
        )
    return in_maps, perms, jp, ip


def run(inputs, trace=False):
    in_maps, perms, jp, ip = prep_inputs(**inputs)
    nc = _get_nc(jp, ip)
    res = run_bass_kernel_spmd(
        nc, in_maps, core_ids=list(range(B)), trace=trace,
        **({"trace_cores": [0]} if trace else {}),
    )
    out = np.empty((B, I, D), dtype=np.float32)
    for b in range(B):
        out[b][perms[b]] = res.results[b]["y"]
    return out, res


def kernel(**inputs):
    out, _ = run(inputs, trace=False)
    return out



# revision 41
# speedup vs baseline: 1.0911x; 1.0911x over previous
"""Cross-attention (txt queries -> image kv) Trainium2 Bass kernel.

Sharding: data-parallel over batch — B=8 batches, one NeuronCore each.
Host-side prep: image columns are COMPACTED to valid kv positions (padded to
jp, a multiple of 128) and txt rows PERMUTED valid-first (attention runs on
the first ip columns only; outputs un-permuted on host). Invalid-q rows are
reconstructed exactly via the ymeanb blend (uniform attention over all kv).
Per core (batch b):
  Q^T = Wq^T T^T / 8          [e, i]   bf16 SBUF-resident
  ONE dense stream over imT (per 512-col block): V' = (X Wv) * kvm_j [j, e]
    SBUF-resident as [j, jc, h, 65] tiles whose col 64 holds kvm_j (so PV
    accumulates both the numerator and the softmax denominator with the kv
    mask applied exactly), and K^T = Wk^T X^T [e, j] SBUF-resident — K and V
    share each image block load; one long back-to-back PE burst keeps the
    HAM clock-gate warm.
  Then attention per head-pair hp (2 heads per psum tile):
    S^T_h = K_h Q_h^T         [j, i]   psum [128,2,512]
    P^T = exp(S^T)            (padded j columns have K^T=0 -> exp(0)=1, and
                               are killed by V'=0 / kvm denominator row)
    O^T_aug_h = [V'_h | kvm]^T P^T -> [65, i] psum accum over j chunks
    O^T = O^T_aug[0:64] * recip(denom) broadcast via PE ones-outer-product
  Y = O^T.T Wout; blend: y = qm_i*Y + (1-qm_i)*ymeanb + qm_i*bout
    (q_mask=False rows = uniform attention over all kv -> host-computed
     ymeanb = (mean_j X) @ Wv @ Wout + bout)

All PE matmuls in bf16 (inputs pre-cast to bf16 on host; intermediates cast
during psum->sbuf copies). Accumulation is fp32 in PSUM throughout. fp8
(e4m3) projection paths exist behind FP8_* flags but are OFF: measured
end-to-end rel-err 1.7e-2 (K) to 3.4e-2 (K+V) vs the 2e-2 gate.

build_nc(reps=N) wraps the whole body in an on-device For_i: test.py times
reps=1 vs reps=33 dispatches and differences them to remove the ~80 ms
axon-relay floor from the reported HW exec time.
"""

import ml_dtypes
import numpy as np

import concourse.bass as bass
from concourse import bacc
import concourse.mybir as mybir
import concourse.tile as tile
from concourse.bass_utils import run_bass_kernel_spmd

B, I, J, D, E = 8, 512, 4096, 1024, 1024
H, DH = 16, 64
JC = J // 128   # 32
IC = I // 128   # 4
DC = D // 128   # 8
EC = E // 128   # 8
F32 = mybir.dt.float32
BF16 = mybir.dt.bfloat16
BF = ml_dtypes.bfloat16

# Structure switch: False = dense sequential phases (KV stream once, then
# attention) — keeps PE streams long for the HAM clock-gate. True = 4-pass
# flash-fused variant (better sim, measured worse on HW).
FUSED = False
# fp8 (e4m3) DoubleRow for the projection matmuls: image/txt activations in
# fp8 directly (values ~N(0,1)); weights pre-scaled by WSCALE=64 on host so
# they sit in e4m3's normal range, descaled when psum is read back.
FP8_KV = False   # K and V projections
FP8_Q = False    # Q projection
D_INWINDOW = True  # SHIPPED: same-window back-to-back A/B measured the
# in-window-D config 28us faster (302.7 vs 330.9 us) on a warm device;
# in-window D partials ride free when the window is throttle-bound and
# they shorten the tail. (On a cold device the two configs tied.): in-window D partials
# steal 's2' psum slots from QK, shrinking ACT's lookahead cushion and
# stretching the attention window ~1:1 (sim 259.8us vs 248.5us)
F8 = mybir.dt.float8e4
NP8 = ml_dtypes.float8_e4m3
WSCALE = 64.0


def build_nc(jp=J, ip=I, reps=1):
    import contextlib

    jcp = jp // 128
    nc = bacc.Bacc()

    txtT = nc.dram_tensor("txtT", [D, I], F8 if FP8_Q else BF16, kind="ExternalInput")
    imT = nc.dram_tensor("imT", [D, jp], F8 if FP8_KV else BF16, kind="ExternalInput")
    wq = nc.dram_tensor("wq", [D, E], F8 if FP8_Q else BF16, kind="ExternalInput")
    wkv = nc.dram_tensor("wkv", [D, 2 * E], F8 if FP8_KV else BF16, kind="ExternalInput")
    wout = nc.dram_tensor("wout", [E, D], BF16, kind="ExternalInput")
    kvmp = nc.dram_tensor("kvmp", [128, jcp], F32, kind="ExternalInput")
    qmp = nc.dram_tensor("qmp", [128, IC], F32, kind="ExternalInput")
    qmrow = nc.dram_tensor("qmrow", [1, I], BF16, kind="ExternalInput")
    omqrow = nc.dram_tensor("omqrow", [1, I], BF16, kind="ExternalInput")
    ymeanb = nc.dram_tensor("ymeanb", [1, D], BF16, kind="ExternalInput")
    boutr = nc.dram_tensor("boutr", [1, D], BF16, kind="ExternalInput")
    y = nc.dram_tensor("y", [I, D], F32, kind="ExternalOutput")
    # V round-trip buffer: [jc, p, h, dh] so phase-B writes are contiguous
    vdr = nc.dram_tensor("vdr", [jcp, 128, H, DH], BF16, kind="Internal")

    imTr = imT[:].rearrange("(dc p) j -> p dc j", p=128)

    with tile.TileContext(nc) as tc:
        with (
            tc.tile_pool(name="wpool", bufs=1) as wpool,      # small resident
            tc.tile_pool(name="wslot", bufs=2) as wslot,      # wk+wv live together in fused pass
            tc.tile_pool(name="big", bufs=1) as big,          # K^T, Q^T, O^T
            tc.tile_pool(name="streamq", bufs=1) as streamq,  # txtT (phase Q only)
            tc.tile_pool(name="stream", bufs=2) as stream,    # imT/vt blocks
            tc.tile_pool(name="work", bufs=3) as work,        # small working tiles
            tc.tile_pool(name="etp", bufs=9) as etp,
            tc.tile_pool(
                name="ppool", bufs=(2 if FUSED else 3), space="PSUM"
            ) as ppool,
            tc.tile_pool(
                name="accp", bufs=(4 if FUSED else 2), space="PSUM"
            ) as accp,
            tc.For_i(
                0, reps, 1,
                hint_engines=(
                    mybir.EngineType.PE,
                    mybir.EngineType.Activation,
                    mybir.EngineType.DVE,
                    mybir.EngineType.Pool,
                    mybir.EngineType.SP,
                ),
            ) if reps > 1 else contextlib.nullcontext(),
        ):
            # big phase-Q inputs FIRST so their transfers head the DMA order
            ta = streamq.tile([128, DC, I], F8 if FP8_Q else BF16, tag="im")
            nc.gpsimd.dma_start(ta[:], txtT[:].rearrange("(dc p) i -> p dc i", p=128))
            wq_sb = wslot.tile([128, DC, E], F8 if FP8_Q else BF16, tag="w")
            nc.gpsimd.dma_start(
                wq_sb[:, :, 0 : E // 2],
                wq[:, 0 : E // 2].rearrange("(dc p) e -> p dc e", p=128),
            )
            nc.gpsimd.dma_start(
                wq_sb[:, :, E // 2 : E],
                wq[:, E // 2 : E].rearrange("(dc p) e -> p dc e", p=128),
            )

            # ---- resident small inputs ----
            kvm_sb = wpool.tile([128, jcp], F32)
            nc.sync.dma_start(kvm_sb[:], kvmp[:])
            qmp_sb = wpool.tile([128, IC], F32)
            nc.sync.dma_start(qmp_sb[:], qmp[:])
            qmrow_sb = wpool.tile([1, I], BF16)
            nc.sync.dma_start(qmrow_sb[:], qmrow[:])
            omqrow_sb = wpool.tile([1, I], BF16)
            nc.sync.dma_start(omqrow_sb[:], omqrow[:])
            ymeanb_sb = wpool.tile([1, D], BF16)
            nc.sync.dma_start(ymeanb_sb[:], ymeanb[:])
            boutr_sb = wpool.tile([1, D], BF16)
            nc.sync.dma_start(boutr_sb[:], boutr[:])
            ones64 = wpool.tile([1, DH], BF16)
            nc.vector.memset(ones64[:], 1.0)
            if FP8_KV:
                kvm_v = wpool.tile([128, jcp], F32)
                nc.vector.tensor_scalar_mul(kvm_v[:], kvm_sb[:], 1.0 / WSCALE)
            else:
                kvm_v = kvm_sb

            KT_sb = big.tile([128, EC, jp], BF16)
            QT_sb = big.tile([128, EC, I], BF16)          # 8 KB/part
            OT_sb = big.tile([128, EC, I], BF16)
            nc.vector.memset(OT_sb[:], 0.0)

            # ============ Phase Q: Q^T = Wq^T @ T^T, scaled 1/8 ==========
            for ep in range(EC // 2):
                ps = ppool.tile([128, 2, 512], F32, tag="s2")
                for eh in range(2):
                    ec = 2 * ep + eh
                    if FP8_Q:
                        for dc in range(0, DC, 2):
                            nc.tensor.matmul(
                                ps[:, eh, 0:ip],
                                wq_sb[:, dc : dc + 2, ec * 128 : (ec + 1) * 128],
                                ta[:, dc : dc + 2, 0:ip],
                                start=(dc == 0),
                                stop=(dc == DC - 2),
                                perf_mode=mybir.MatmulPerfMode.DoubleRow,
                            )
                    else:
                        for dc in range(DC):
                            nc.tensor.matmul(
                                ps[:, eh, 0:ip],
                                wq_sb[:, dc, ec * 128 : (ec + 1) * 128],
                                ta[:, dc, 0:ip],
                                start=(dc == 0),
                                stop=(dc == DC - 1),
                            )
                nc.vector.tensor_scalar_mul(
                    QT_sb[:, 2 * ep : 2 * ep + 2, 0:ip],
                    ps[:, :, 0:ip],
                    0.125 / (WSCALE if FP8_Q else 1.0),
                )

            # Padded j columns have K^T = 0 (image compacted+zero-padded) so
            # S = 0 there and exp(0)=1; both numerator and denominator kill
            # them via V'=0 / kvm row. No mask needed inside exp.
            v_res = jp <= 2560

            jblocks = []
            off = 0
            while off < jp:
                w = 512 if jp - off >= 512 else jp - off
                jblocks.append((off, w))
                off += w

            def epilogue(hp, hh, oacc):
                # 1/denom broadcast along partitions via the GPSIMD
                # partition_broadcast custom op — Pool is idle during
                # attention and this keeps the epilogue entirely off PE.
                rec = work.tile([1, ip], F32, tag="rec")
                nc.vector.reciprocal(rec[:, 0:ip], oacc[DH : DH + 1, 0:ip])
                rb = work.tile([DH, ip], F32, tag="rb")
                nc.gpsimd.partition_broadcast(rb[:, 0:ip], rec[:, 0:ip])
                nc.vector.tensor_tensor(
                    OT_sb[hh * DH : (hh + 1) * DH, hp, 0:ip],
                    oacc[0:DH, 0:ip],
                    rb[:, 0:ip],
                    mybir.AluOpType.mult,
                )

            def qk_exp(hp, jc):
                sps = ppool.tile([128, 2, 512], F32, tag="s2")
                nc.tensor.matmul(
                    sps[:, 0, 0:ip],
                    KT_sb[0:DH, hp, jc * 128 : (jc + 1) * 128],
                    QT_sb[0:DH, hp, 0:ip],
                    start=True,
                    stop=True,
                )
                nc.tensor.matmul(
                    sps[:, 1, 0:ip],
                    KT_sb[DH:128, hp, jc * 128 : (jc + 1) * 128],
                    QT_sb[DH:128, hp, 0:ip],
                    start=True,
                    stop=True,
                )
                et = etp.tile([128, 2, ip], BF16, tag="et")
                nc.scalar.activation(
                    et[:, :, 0:ip],
                    sps[:, :, 0:ip],
                    mybir.ActivationFunctionType.Exp,
                )
                return et

            def pv(jc, vt, et, oacc_a, oacc_b):
                nc.tensor.matmul(
                    oacc_a[:, 0:ip],
                    vt[:, jc, 0, :],
                    et[:, 0, 0:ip],
                    start=(jc == 0),
                    stop=(jc == jcp - 1),
                )
                nc.tensor.matmul(
                    oacc_b[:, 0:ip],
                    vt[:, jc, 1, :],
                    et[:, 1, 0:ip],
                    start=(jc == 0),
                    stop=(jc == jcp - 1),
                )

            def qk_exp_pv(hp, jc, vt, oacc_a, oacc_b):
                et = qk_exp(hp, jc)
                pv(jc, vt, et, oacc_a, oacc_b)

            def k_block(ep, imb, off, w):
                ps = ppool.tile([128, 2, 512], F32, tag="s2")
                for eh in range(2):
                    ec = 2 * ep + eh
                    if FP8_KV:
                        for dc in range(0, DC, 2):
                            nc.tensor.matmul(
                                ps[:, eh, 0:w],
                                wk_sb[:, dc : dc + 2, ec * 128 : (ec + 1) * 128],
                                imb[:, dc : dc + 2, 0:w],
                                start=(dc == 0),
                                stop=(dc == DC - 2),
                                perf_mode=mybir.MatmulPerfMode.DoubleRow,
                            )
                    else:
                        for dc in range(DC):
                            nc.tensor.matmul(
                                ps[:, eh, 0:w],
                                wk_sb[:, dc, ec * 128 : (ec + 1) * 128],
                                imb[:, dc, 0:w],
                                start=(dc == 0),
                                stop=(dc == DC - 1),
                            )
                if FP8_KV:
                    nc.vector.tensor_scalar_mul(
                        KT_sb[:, 2 * ep : 2 * ep + 2, off : off + w],
                        ps[:, :, 0:w],
                        1.0 / WSCALE,
                    )
                else:
                    nc.vector.tensor_copy(
                        KT_sb[:, 2 * ep : 2 * ep + 2, off : off + w], ps[:, :, 0:w]
                    )

            def v_block(imb, off, w):
                for jh in range(w // 128):
                    jc = off // 128 + jh
                    ps = ppool.tile([128, 2, 512], F32, tag="s2")
                    for eb in range(2):
                        if FP8_KV:
                            for dc in range(0, DC, 2):
                                nc.tensor.matmul(
                                    ps[:, eb, :],
                                    imb[:, dc : dc + 2, jh * 128 : (jh + 1) * 128],
                                    wv_sb[:, dc : dc + 2, eb * 512 : (eb + 1) * 512],
                                    start=(dc == 0),
                                    stop=(dc == DC - 2),
                                    perf_mode=mybir.MatmulPerfMode.DoubleRow,
                                )
                        else:
                            for dc in range(DC):
                                nc.tensor.matmul(
                                    ps[:, eb, :],
                                    imb[:, dc, jh * 128 : (jh + 1) * 128],
                                    wv_sb[:, dc, eb * 512 : (eb + 1) * 512],
                                    start=(dc == 0),
                                    stop=(dc == DC - 1),
                                )
                    if v_res:
                        nc.vector.tensor_scalar_mul(
                            V_sb[:, jc, :, 0:DH],
                            ps[:].rearrange("p b (h dh) -> p (b h) dh", dh=DH),
                            kvm_v[:, jc : jc + 1],
                        )
                    else:
                        vtmp = work.tile([128, H, DH], BF16, tag="vtmp")
                        nc.vector.tensor_scalar_mul(
                            vtmp[:],
                            ps[:].rearrange("p b (h dh) -> p (b h) dh", dh=DH),
                            kvm_v[:, jc : jc + 1],
                        )
                        nc.sync.dma_start(vdr[jc, :, :, :], vtmp[:])

            if v_res and not FUSED:
                # === v2: dense sequential phases. ONE stream over imT
                # producing V (resident) + all of K^T back-to-back (long PE
                # burst, HAM-friendly), then attention per head-pair.
                # (Measured best on HW: 309.9us vs 326.4us for a two-stream
                # K/attention pipeline and 363.1us for per-block fusion.) ===
                V_sb = big.tile([128, jcp, H, DH + 1], BF16)
                nc.vector.tensor_copy(
                    V_sb[:, :, :, DH : DH + 1],
                    kvm_sb[:, :, None, None].to_broadcast([128, jcp, H, 1]),
                )
                imb0 = stream.tile([128, DC, 512], F8 if FP8_KV else BF16, tag="ima")
                nc.gpsimd.dma_start(
                    imb0[:, :, 0 : jblocks[0][1]],
                    imTr[:, :, 0 : jblocks[0][1]],
                )
                wv_sb = wslot.tile([128, DC, E], F8 if FP8_KV else BF16, tag="w")
                nc.gpsimd.dma_start(
                    wv_sb[:], wkv[:, E : 2 * E].rearrange("(dc p) e -> p dc e", p=128)
                )
                wk_sb = wslot.tile([128, DC, E], F8 if FP8_KV else BF16, tag="w")
                nc.gpsimd.dma_start(
                    wk_sb[:], wkv[:, 0:E].rearrange("(dc p) e -> p dc e", p=128)
                )
                for bi, (off, w) in enumerate(jblocks):
                    if bi == 0:
                        imb = imb0
                    else:
                        imb = stream.tile([128, DC, 512], F8 if FP8_KV else BF16, tag="ima")
                        nc.gpsimd.dma_start(
                            imb[:, :, 0:w], imTr[:, :, off : off + w]
                        )
                    v_block(imb, off, w)
                    for ep in range(EC // 2):
                        k_block(ep, imb, off, w)
                icv = (ip + 127) // 128
                if D_INWINDOW:
                    wo_sb = wslot.tile([128, DC, E], BF16, tag="w")
                    nc.scalar.dma_start(
                        wo_sb[:], wout[:].rearrange("(ec p) d -> p ec d", p=128)
                    )
                    y_acc = big.tile([128, icv, 2, 512], F32)

                def d_partial_ic(hp, ic):
                    ps = ppool.tile([128, 2, 512], F32, tag="s2")
                    for db in range(2):
                        nc.tensor.matmul(
                            ps[:, db, :],
                            OT_sb[:, hp, ic * 128 : (ic + 1) * 128],
                            wo_sb[:, hp, db * 512 : (db + 1) * 512],
                            start=True,
                            stop=True,
                        )
                    if hp == 0:
                        nc.vector.tensor_copy(y_acc[:, ic, :, :], ps[:])
                    else:
                        nc.vector.tensor_tensor(
                            y_acc[:, ic, :, :],
                            ps[:],
                            y_acc[:, ic, :, :],
                            mybir.AluOpType.add,
                        )

                # epilogue/d_partial for head-pair hp are EMITTED a few jc
                # iterations into head-pair hp+1 so the PE queue never
                # head-of-line blocks on the epilogue's DVE chain — ACT keeps
                # streaming exps across the boundary.
                # Deferred work from head-pair hp-1 (epilogues, phase-D
                # partial out-projections) is dripped into hp's jc loop one
                # piece every other iteration: each dent in the PE queue must
                # stay under the ~2-slot sps cushion or ACT's exp stream
                # hiccups. PV trails QK/exp by one step, carried across
                # boundaries, so the in-order PE queue never blocks on exp.
                pend = []
                prev = None  # (jc, vt, et, oacc_a, oacc_b)
                for hp in range(EC):
                    vt = V_sb[:, :, 2 * hp : 2 * hp + 2, :]
                    oacc_a = accp.tile([DH + 1, 512], F32, tag="oacc")
                    oacc_b = accp.tile([DH + 1, 512], F32, tag="oacc")
                    for jc in range(jcp):
                        et = qk_exp(hp, jc)
                        if prev is not None:
                            pv(*prev)
                        prev = (jc, vt, et, oacc_a, oacc_b)
                        if jc >= 3 and jc % 2 == 1 and pend:
                            pend.pop(0)()
                    # drain leftovers (only fires for tiny jcp) before
                    # queueing this head-pair's deferred work
                    for f in pend:
                        f()
                    ph, pa, pb = hp, oacc_a, oacc_b
                    pend = [
                        lambda ph=ph, pa=pa: epilogue(ph, 0, pa),
                        lambda ph=ph, pb=pb: epilogue(ph, 1, pb),
                    ]
                    if D_INWINDOW:
                        pend += [
                            lambda ph=ph: d_partial_ic(ph, 0),
                            lambda ph=ph: d_partial_ic(ph, 1),
                            lambda ph=ph: d_partial_ic(ph, 2),
                        ][:icv]
                pv(*prev)
                for f in pend:
                    f()
            elif v_res:
                # === v3: 4 fused passes over imT. Pass p computes K^T for ec
                # chunks {2p, 2p+1} (+ V on pass 0) and, block by block, runs
                # QK->exp->PV for head-pairs 2p and 2p+1; exp (ACT) hides
                # under the projection matmuls (PE). PV accumulates in PSUM
                # across the whole stream (4 banks live per pass).
                V_sb = big.tile([128, jcp, H, DH + 1], BF16)
                nc.vector.tensor_copy(
                    V_sb[:, :, :, DH : DH + 1],
                    kvm_sb[:, :, None, None].to_broadcast([128, jcp, H, 1]),
                )
                # block-0 image lands before wv/wk so pass-0 V can start early
                imb0 = stream.tile([128, DC, 512], F8 if FP8_KV else BF16, tag="ima")
                nc.gpsimd.dma_start(
                    imb0[:, :, 0 : jblocks[0][1]],
                    imTr[:, :, 0 : jblocks[0][1]],
                )
                wv_sb = wslot.tile([128, DC, E], F8 if FP8_KV else BF16, tag="w")
                nc.gpsimd.dma_start(
                    wv_sb[:], wkv[:, E : 2 * E].rearrange("(dc p) e -> p dc e", p=128)
                )
                wk_sb = wslot.tile([128, DC, E], F8 if FP8_KV else BF16, tag="w")
                nc.gpsimd.dma_start(
                    wk_sb[:], wkv[:, 0:E].rearrange("(dc p) e -> p dc e", p=128)
                )
                for p in range(EC // 2):
                    oaccs = [
                        [
                            accp.tile(
                                [DH + 1, 512], F32, tag="oacc",
                                name=f"oacc_{p}_{hl}_{hh}",
                            )
                            for hh in range(2)
                        ]
                        for hl in range(2)
                    ]  # [hp_local][hh]

                    def attn_block(off, w):
                        for hl in range(2):
                            hp = 2 * p + hl
                            vt = V_sb[:, :, 2 * hp : 2 * hp + 2, :]
                            for jh in range(w // 128):
                                qk_exp_pv(
                                    hp,
                                    off // 128 + jh,
                                    vt,
                                    oaccs[hl][0],
                                    oaccs[hl][1],
                                )

                    # one-block software pipeline: attention for block b-1
                    # runs while block b's K projection streams, hiding the
                    # psum->KT copy latency.
                    prev = None
                    for bi, (off, w) in enumerate(jblocks):
                        if p == 0 and bi == 0:
                            imb = imb0
                        else:
                            imb = stream.tile([128, DC, 512], F8 if FP8_KV else BF16, tag="ima")
                            nc.gpsimd.dma_start(
                                imb[:, :, 0:w], imTr[:, :, off : off + w]
                            )
                        if p == 0:
                            v_block(imb, off, w)
                        k_block(p, imb, off, w)
                        if prev is not None:
                            attn_block(*prev)
                        prev = (off, w)
                    attn_block(*prev)
                    for hl in range(2):
                        for hh in range(2):
                            epilogue(2 * p + hl, hh, oaccs[hl][hh])
            else:
                # === fallback (large jp): single fused K+V pass with V via
                # DRAM round-trip, then attention per head-pair ===
                wv_sb = wslot.tile([128, DC, E], F8 if FP8_KV else BF16, tag="w")
                nc.gpsimd.dma_start(
                    wv_sb[:], wkv[:, E : 2 * E].rearrange("(dc p) e -> p dc e", p=128)
                )
                wk_sb = wslot.tile([128, DC, E], F8 if FP8_KV else BF16, tag="w")
                nc.gpsimd.dma_start(
                    wk_sb[:], wkv[:, 0:E].rearrange("(dc p) e -> p dc e", p=128)
                )
                for off, w in jblocks:
                    imb = stream.tile([128, DC, 512], F8 if FP8_KV else BF16, tag="ima")
                    nc.gpsimd.dma_start(imb[:, :, 0:w], imTr[:, :, off : off + w])
                    v_block(imb, off, w)
                    for ep in range(EC // 2):
                        k_block(ep, imb, off, w)
                for hp in range(EC):
                    vtt = stream.tile([128, jcp, 2, DH + 1], BF16, tag="vt")
                    nc.vector.tensor_copy(
                        vtt[:, :, 0, DH : DH + 1], kvm_sb[:, :, None]
                    )
                    nc.vector.tensor_copy(
                        vtt[:, :, 1, DH : DH + 1], kvm_sb[:, :, None]
                    )
                    for hh in range(2):
                        nc.sync.dma_start(
                            vtt[:, :, hh, 0:DH],
                            vdr[:, :, 2 * hp + hh, :].rearrange("jc p dh -> p jc dh"),
                        )
                    oacc_a = accp.tile([DH + 1, 512], F32, tag="oacc")
                    oacc_b = accp.tile([DH + 1, 512], F32, tag="oacc")
                    for jc in range(jcp):
                        qk_exp_pv(hp, jc, vtt, oacc_a, oacc_b)
                    epilogue(hp, 0, oacc_a)
                    epilogue(hp, 1, oacc_b)

            # ============ Phase D tail: blend + store ====================
            have_yacc = v_res and not FUSED and D_INWINDOW
            if not have_yacc:
                wo_sb = wslot.tile([128, DC, E], BF16, tag="w")
                nc.scalar.dma_start(
                    wo_sb[:], wout[:].rearrange("(ec p) d -> p ec d", p=128)
                )
            for ic in range(IC):
                has_valid = ic * 128 < ip
                if has_valid and not have_yacc:
                    yps = ppool.tile([128, 2, 512], F32, tag="s2")
                    for db in range(2):
                        for ec in range(EC):
                            nc.tensor.matmul(
                                yps[:, db, :],
                                OT_sb[:, ec, ic * 128 : (ic + 1) * 128],
                                wo_sb[:, ec, db * 512 : (db + 1) * 512],
                                start=(ec == 0),
                                stop=(ec == EC - 1),
                            )
                bb_a = accp.tile([128, 512], F32, tag="oacc")
                bb_b = accp.tile([128, 512], F32, tag="oacc")
                for db, bbps in ((0, bb_a), (1, bb_b)):
                    nc.tensor.matmul(
                        bbps[:],
                        omqrow_sb[:, ic * 128 : (ic + 1) * 128],
                        ymeanb_sb[:, db * 512 : (db + 1) * 512],
                        start=True,
                        stop=False,
                    )
                    nc.tensor.matmul(
                        bbps[:],
                        qmrow_sb[:, ic * 128 : (ic + 1) * 128],
                        boutr_sb[:, db * 512 : (db + 1) * 512],
                        start=False,
                        stop=True,
                    )
                y1 = work.tile([128, 2, 512], F32, tag="y1")
                if has_valid:
                    nc.vector.tensor_scalar_mul(
                        y1[:],
                        y_acc[:, ic, :, :] if have_yacc else yps[:],
                        qmp_sb[:, ic : ic + 1],
                    )
                    nc.vector.tensor_tensor(
                        y1[:, 0, :], bb_a[:], y1[:, 0, :], mybir.AluOpType.add
                    )
                    nc.vector.tensor_tensor(
                        y1[:, 1, :], bb_b[:], y1[:, 1, :], mybir.AluOpType.add
                    )
                else:
                    nc.vector.tensor_copy(y1[:, 0, :], bb_a[:])
                    nc.vector.tensor_copy(y1[:, 1, :], bb_b[:])
                nc.sync.dma_start(
                    y[ic * 128 : (ic + 1) * 128, :],
                    y1[:].rearrange("p b d -> p (b d)"),
                )

    nc.compile()
    return nc


_NC_CACHE = {}


def _get_nc(jp=J, ip=I, reps=1):
    key = (jp, ip, reps)
    if key not in _NC_CACHE:
        _NC_CACHE[key] = build_nc(jp, ip, reps)
    return _NC_CACHE[key]


def prep_inputs(txt, image, kv_mask, q_mask, Wq, Wkv, Wout, bout):
    f32 = np.float32
    Wq = np.asarray(Wq, dtype=f32)
    Wkv = np.asarray(Wkv, dtype=f32)
    Wout = np.asarray(Wout, dtype=f32)
    bout = np.asarray(bout, dtype=f32)
    wq_b = Wq.astype(BF)
    wkv_b = Wkv.astype(BF)
    wout_b = Wout.astype(BF)
    kvc = kv_mask.sum(axis=1).max()
    qc = q_mask.sum(axis=1).max()
    jp = max(512, int(-(-kvc // 128)) * 128)
    ip = max(256, int(-(-qc // 16)) * 16)
    jcp = jp // 128
    in_maps = []
    perms = []
    for b in range(B):
        kvm = kv_mask[b].astype(bool)
        qm = q_mask[b].astype(bool)
        nkv = int(kvm.sum())
        # compact image columns to valid kv positions, zero-pad to jp
        imTc = np.zeros((D, jp), dtype=BF)
        imTc[:, :nkv] = np.ascontiguousarray(image[b][kvm].T).astype(BF)
        kvmp = np.zeros(jp, dtype=f32)
        kvmp[:nkv] = 1.0
        # permute txt rows valid-first
        perm = np.argsort(~qm, kind="stable")
        perms.append(perm)
        qmperm = qm[perm].astype(f32)
        xmean = image[b].astype(f32).mean(axis=0)
        vmean = xmean @ Wkv[:, E:]
        ymb = vmean @ Wout + bout
        in_maps.append(
            {
                "txtT": np.ascontiguousarray(txt[b][perm].T).astype(BF),
                "imT": imTc,
                "wq": wq_b,
                "wkv": wkv_b,
                "wout": wout_b,
                "kvmp": np.ascontiguousarray(kvmp.reshape(jcp, 128).T),
                "qmp": np.ascontiguousarray(qmperm.reshape(IC, 128).T),
                "qmrow": qmperm[None, :].astype(BF),
                "omqrow": (1.0 - qmperm)[None, :].astype(BF),
                "ymeanb": ymb[None, :].astype(BF),
                "boutr": bout[None, :].astype(BF),
            }
        )
    return in_maps, perms, jp, ip


def run(inputs, trace=False):
    in_maps, perms, jp, ip = prep_inputs(**inputs)
    nc = _get_nc(jp, ip)
    res = run_bass_kernel_spmd(
        nc, in_maps, core_ids=list(range(B)), trace=trace,
        **({"trace_cores": [0]} if trace else {}),
    )
    out = np.empty((B, I, D), dtype=np.float32)
    for b in range(B):
        out[b][perms[b]] = res.results[b]["y"]
    return out, res


def kernel(**inputs):
    out, _ = run(inputs, trace=False)
    return out

